# revision 50
# baseline (speedup 1.0000x reference)
"""Trainium2 Bass kernel for nn_Block_Attention_9225589752303.

Sharding: 8 cores = 4 batches x 2 image-row halves (data parallel; attention is
independent per (batch, patch) except the Q_block reduction, AllReduced over
core pairs).

Host-side prep materializes the scrambled patchify views (pure data movement,
part of sharding) so the on-chip kernel is a pure matmul/softmax pipeline:

  inc   = relu(BN(inc_W @ x[:, 8h:8h+8, :]))  -> 4x4 avg+max pool -> inc_flat
  Q     = Wq @ P                (3-term bf16 hi/lo; consumed from PSUM)
  Qb    = sum_n Q_n * inc_flat[:, n] + bq * sum(inc_flat)   (+AllReduce)
  M2    = Wk^T @ Qb             (fp32 matmul, exact; K never materialized)
  S^T   = P^T M2 + 1*c_off      (3-term bf16; c_off = Qb^T bk broadcast)
  A^T   = softmax_free(S^T) -> block-diag pairs -> A  (PE transpose)
  Z     = P @ A                 (bf16; 2 patches per matmul via block-diag A)
  out   = Wv @ Z + (x + bv)     (bf16 matmul, bf16 residual/output)

The score path needs ~22-bit operands (logits reach ~1e3, softmax-sensitive),
hence 3-term hi/lo; fp32r (11-bit internal rounding) was measured too noisy.
The Z/Wv path runs in bf16 (rel err ~5e-3 vs fp32 reference).
"""

import sys

import numpy as np

sys.path.insert(0, "/opt/trn_rl_repo")

import ml_dtypes

import concourse.bass as bass
import concourse.bacc as bacc
import concourse.mybir as mybir
from concourse import tile
from concourse._compat import with_exitstack
from concourse.bass_utils import run_bass_kernel_spmd

BF16 = ml_dtypes.bfloat16
F32 = mybir.dt.float32
DBF = mybir.dt.bfloat16
AF = mybir.ActivationFunctionType
ALU = mybir.AluOpType
AX = mybir.AxisListType

C = 2048
N_CORES = 8


def build_program():
    nc = bacc.Bacc()

    ph = nc.declare_dram_parameter("ph", [4, 128, 8192], DBF, isOutput=False)
    pl = nc.declare_dram_parameter("pl", [4, 128, 8192], DBF, isOutput=False)
    phl2t = nc.declare_dram_parameter("phl2t", [16, 128, 4096], DBF, isOutput=False)
    xih = nc.declare_dram_parameter("xih", [128, 8192], DBF, isOutput=False)
    xil = nc.declare_dram_parameter("xil", [128, 8192], DBF, isOutput=False)
    wq2 = nc.declare_dram_parameter("wq2", [128, 8192], DBF, isOutput=False)
    wi2 = nc.declare_dram_parameter("wi2", [128, 8192], DBF, isOutput=False)
    wk3 = nc.declare_dram_parameter("wk3", [128, 4096], F32, isOutput=False)
    wv2 = nc.declare_dram_parameter("wv2", [128, 32768], DBF, isOutput=False)
    pt2 = nc.declare_dram_parameter("pt2", [4, 128, 8192], DBF, isOutput=False)
    xr2 = nc.declare_dram_parameter("xr2", [4, 128, 8192], DBF, isOutput=False)
    bk = nc.declare_dram_parameter("bk", [256], F32, isOutput=False)
    bq = nc.declare_dram_parameter("bq", [256], F32, isOutput=False)
    bns = nc.declare_dram_parameter("bns", [256], F32, isOutput=False)
    bnt = nc.declare_dram_parameter("bnt", [256], F32, isOutput=False)
    idn = nc.declare_dram_parameter("idn", [128, 128], DBF, isOutput=False)
    idn32 = nc.declare_dram_parameter("idn32", [64, 64], F32, isOutput=False)
    out = nc.declare_dram_parameter("out", [128, 32768], DBF, isOutput=True)

    with tile.TileContext(nc) as tc:
        _body(tc, ph=ph, pl=pl, phl2t=phl2t, xih=xih, xil=xil,
              wq2=wq2, wi2=wi2, wk3=wk3, wv2=wv2, pt2=pt2, xr2=xr2, bk=bk,
              bq=bq, bns=bns, bnt=bnt, idn=idn, idn32=idn32, out=out)
    nc.compile()
    return nc


@with_exitstack
def _body(ctx, tc, *, ph, pl, phl2t, xih, xil, wq2, wi2, wk3, wv2,
          pt2, xr2, bk, bq, bns, bnt, idn, idn32, out):
    nc = tc.nc

    # ---------------- long-lived tiles ----------------
    mid = ctx.enter_context(tc.tile_pool(name="mid", bufs=1))
    qb = mid.tile([128, 2, 64], F32, tag="qb", name="qb")            # Q_block
    qbp = mid.tile([128, 4, 2, 64], F32, tag="qbp", name="qbp")      # partials
    incflat = mid.tile([128, 2, 32], F32, tag="incflat", name="incflat")
    # packed per-partition vectors: bk(0:2) bq(2:4) bns(4:6) bnt(6:8) corr(8:10)
    smalls = mid.tile([128, 10], F32, tag="smalls", name="smalls")
    bk_t, bq_t = smalls[:, 0:2], smalls[:, 2:4]
    bns_t, bnt_t = smalls[:, 4:6], smalls[:, 6:8]
    corr = smalls[:, 8:10]
    idn_t = mid.tile([128, 128], DBF, tag="idn_t", name="idn_t")
    idn32_t = mid.tile([64, 64], F32, tag="idn32_t", name="idn32_t")
    # block-diag A per patch pair: [x-pair, pair, y-pair]
    a_blk = mid.tile([128, 16, 128], DBF, tag="a_blk", name="a_blk")
    # M2 = Wk^T Qb as bf16 hi/lo: [c-part, c-slice, x]
    m2h = mid.tile([128, 16, 64], DBF, tag="m2h", name="m2h")
    m2l = mid.tile([128, 16, 64], DBF, tag="m2l", name="m2l")
    wk3_t = mid.tile([128, 2, 16, 128], F32, tag="wk3", name="wk3")
    cf_t = mid.tile([1, 64], F32, tag="cf", name="cf")
    ones1 = mid.tile([1, 128], F32, tag="ones1", name="ones1")

    # ================= phase 1: inc branch + Q projection ===================
    dramp = ctx.enter_context(tc.tile_pool(name="dramp", bufs=1, space="DRAM"))
    with (
        tc.tile_pool(name="wkq", bufs=1) as wkq_pool,
        tc.tile_pool(name="php", bufs=2) as php,
        tc.tile_pool(name="plp", bufs=2) as plp,
        tc.tile_pool(name="str1", bufs=2) as str1,
        tc.tile_pool(name="psA", bufs=2, space="PSUM") as psA,
    ):
        wq_t = wkq_pool.tile([128, 16, 2, 256], DBF, tag="wq_t", name="wq_t")
        with tc.tile_pool(name="xiw", bufs=1) as xiw:
            wi_t = xiw.tile([128, 16, 2, 256], DBF, tag="wi_t", name="wi_t")
            xih_t = xiw.tile([128, 16, 512], DBF, tag="xih", name="xih")
            xil_t = xiw.tile([128, 16, 512], DBF, tag="xil", name="xil")

            nc.sync.dma_start(out=wi_t[:, :, :, :],
                              in_=wi2.rearrange("q (c t k) -> q c t k", c=16, t=2))
            nc.sync.dma_start(out=xih_t[:, :, :],
                              in_=xih.rearrange("q (c n) -> q c n", c=16))
            nc.sync.dma_start(out=xil_t[:, :, :],
                              in_=xil.rearrange("q (c n) -> q c n", c=16))
            for i, vec in enumerate((bk, bq, bns, bnt)):
                nc.sync.dma_start(out=smalls[:, 2 * i:2 * i + 2],
                                  in_=vec.rearrange("(j q) -> q j", q=128))
            nc.sync.dma_start(out=idn_t[:, :], in_=idn[:, :])
            nc.sync.dma_start(out=idn32_t[:, :], in_=idn32[:, :])
            nc.sync.dma_start(out=wq_t[:, :, :, :],
                              in_=wq2.rearrange("q (c t k) -> q c t k", c=16, t=2))
            nc.vector.memset(ones1[:, :], 1.0)

            # ---- incidence branch (3-term hi/lo) ----
            incps = [psA.tile([128, 512], F32, tag="incps", name="incps")
                     for _ in range(2)]
            for kc in range(16):
                for jc in range(2):
                    for ti, (wt, xt) in enumerate(
                            [(0, xih_t), (0, xil_t), (1, xih_t)]):
                        nc.tensor.matmul(
                            incps[jc][:, :],
                            wi_t[:, kc, wt, 128 * jc:128 * (jc + 1)],
                            xt[:, kc, :],
                            start=(kc == 0 and ti == 0),
                            stop=(kc == 15 and ti == 2))
        for jc in range(2):
            inc_sb = str1.tile([128, 512], F32, tag="incsb", name="incsb")
            # relu(raw * s + t)  (BN folded host-side)
            nc.scalar.activation(inc_sb[:, :], incps[jc][:, :], AF.Relu,
                                 bias=bnt_t[:, jc:jc + 1], scale=bns_t[:, jc:jc + 1])
            # 4x4 avg+max pool: cols = (ph2, h4, pw16, w4) strides (256,64,4,1)
            v = inc_sb.rearrange("q (ph h pw w) -> q ph pw h w", ph=2, h=4, pw=16)
            psum_t = str1.tile([128, 32], F32, tag="poolsum", name="poolsum", bufs=1)
            pmax_t = str1.tile([128, 32], F32, tag="poolmax", name="poolmax", bufs=1)
            ps4 = psum_t.rearrange("q (ph pw) -> q ph pw", ph=2)
            pm4 = pmax_t.rearrange("q (ph pw) -> q ph pw", ph=2)
            nc.vector.tensor_reduce(ps4, v, axis=AX.XY, op=ALU.add)
            nc.vector.tensor_reduce(pm4, v, axis=AX.XY, op=ALU.max)
            # incflat = sum/16 + max
            nc.vector.scalar_tensor_tensor(
                incflat[:, jc, :], psum_t[:, :], 1.0 / 16.0, pmax_t[:, :],
                op0=ALU.mult, op1=ALU.add)
            incsum = str1.tile([128, 1], F32, tag="incsum", name="incsum", bufs=1)
            nc.vector.tensor_reduce(incsum[:, :], incflat[:, jc, :], axis=AX.X, op=ALU.add)
            nc.vector.tensor_tensor(corr[:, jc:jc + 1], bq_t[:, jc:jc + 1],
                                    incsum[:, :], op=ALU.mult)

        # ---- Q pass over patch chunks (3-term hi/lo); ph stays resident ----
        ph_r = ph.rearrange("b q (c n) -> b q c n", c=16)
        pl_r = pl.rearrange("b q (c n) -> b q c n", c=16)
        qbl = [dramp.tile([128, 2, 64], F32, name="qbl0")]
        qbs = [dramp.tile([128, 2, 64], F32, name="qbs0")]
        for nb in range(4):
            ph_t = php.tile([128, 16, 512], DBF, tag="ph", name="ph")
            pl_t = plp.tile([128, 16, 512], DBF, tag="pl", name="pl")
            nc.sync.dma_start(out=ph_t[:, :, :], in_=ph_r[nb])
            nc.sync.dma_start(out=pl_t[:, :, :], in_=pl_r[nb])
            if nb == 0:
                nc.sync.dma_start(
                    out=wk3_t[:, :, :, :],
                    in_=wk3.rearrange("q (t c k) -> q t c k", t=2, c=16))
            qps = [psA.tile([128, 512], F32, tag="qps", name="qps", bufs=3)
                   for _ in range(2)]
            for kc in range(16):
                for jc in range(2):
                    for ti, (wt, xt) in enumerate(
                            [(0, ph_t), (0, pl_t), (1, ph_t)]):
                        nc.tensor.matmul(
                            qps[jc][:, :],
                            wq_t[:, kc, wt, 128 * jc:128 * (jc + 1)],
                            xt[:, kc, :],
                            start=(kc == 0 and ti == 0),
                            stop=(kc == 15 and ti == 2))
            # per-chunk Qb partial += Q * inc scalars (8 patches per chunk)
            for jc in range(2):
                for li in range(8):
                    l = 8 * nb + li
                    sl = qps[jc][:, 64 * li:64 * (li + 1)]
                    sc = incflat[:, jc, l:l + 1]
                    if li == 0:
                        nc.vector.tensor_scalar(qbp[:, nb, jc, :], sl, sc,
                                                None, op0=ALU.mult)
                    else:
                        nc.vector.scalar_tensor_tensor(
                            qbp[:, nb, jc, :], sl, sc, qbp[:, nb, jc, :],
                            op0=ALU.mult, op1=ALU.add)
            if nb == 3:
                for jc in range(2):
                    nc.vector.tensor_scalar_add(
                        qbp[:, 3, jc, :], qbp[:, 3, jc, :], corr[:, jc:jc + 1])
        # combine the 4 partials, then one pair-AllReduce
        nc.vector.tensor_tensor(qbp[:, 0, :, :], qbp[:, 0, :, :],
                                qbp[:, 1, :, :], op=ALU.add)
        nc.vector.tensor_tensor(qbp[:, 2, :, :], qbp[:, 2, :, :],
                                qbp[:, 3, :, :], op=ALU.add)
        nc.vector.tensor_tensor(qbp[:, 0, :, :], qbp[:, 0, :, :],
                                qbp[:, 2, :, :], op=ALU.add)
        nc.gpsimd.dma_start(out=qbl[0][:, :, :], in_=qbp[:, 0, :, :])
        nc.gpsimd.collective_compute(
            "AllReduce", ALU.add,
            replica_groups=[[0, 1], [2, 3], [4, 5], [6, 7]],
            ins=[qbl[0].opt()], outs=[qbs[0].opt()])
        nc.gpsimd.dma_start(out=qb[:, :, :], in_=qbs[0][:, :, :])

    # wv/pt pools opened early: their DMAs (gpsimd queue) overlap phase 2
    wv_pool = ctx.enter_context(tc.tile_pool(name="wvp", bufs=1))
    ptp = ctx.enter_context(tc.tile_pool(name="ptp", bufs=2))
    wv_t = wv_pool.tile([128, 16, 16, 128], DBF, tag="wv", name="wv")
    pt_r = pt2.rearrange("g q (a pr c) -> g q a pr c", a=16, pr=4)
    pt0 = ptp.tile([128, 16, 4, 128], DBF, tag="pt", name="pt")
    nc.gpsimd.dma_start(out=pt0[:, :, :, :], in_=pt_r[0])
    nc.gpsimd.dma_start(out=wv_t[:, :, :, :],
                        in_=wv2.rearrange("q (a m k) -> q a m k", a=16, m=16))

    # ================= phase 2: M2, scores, softmax -> block-diag A =========
    with (
        tc.tile_pool(name="str2", bufs=4) as str2,
        tc.tile_pool(name="atf", bufs=1) as atfp,
        tc.tile_pool(name="p2p", bufs=3) as p2p,
        tc.tile_pool(name="psB", bufs=4, space="PSUM") as psB,
    ):
        # M2 = Wk^T Qb  (fp32 matmuls, exact), split to bf16 hi/lo
        for cs in range(16):
            m2ps = psB.tile([128, 64], F32, tag="m2ps", name="m2ps", bufs=1)
            for kc in range(2):
                nc.tensor.matmul(
                    m2ps[:, :],
                    wk3_t[:, kc, cs, :],
                    qb[:, kc, :],
                    start=(kc == 0), stop=(kc == 1))
            nc.vector.tensor_copy(m2h[:, cs, :], m2ps[:, :])
            nc.vector.tensor_tensor(m2l[:, cs, :], m2ps[:, :], m2h[:, cs, :],
                                    op=ALU.subtract)
        # c_off[x] = Qb^T bk, transposed onto one partition
        cop = psB.tile([64, 1], F32, tag="cop", name="cop", bufs=1)
        for kc in range(2):
            nc.tensor.matmul(cop[:, :], qb[:, kc, :], bk_t[:, kc:kc + 1],
                             start=(kc == 0), stop=(kc == 1))
        cosb = str2.tile([64, 1], F32, tag="cosb", name="cosb")
        nc.vector.tensor_copy(cosb[:, :], cop[:, :])
        coT = psB.tile([1, 64], F32, tag="coT", name="coT", bufs=1)
        nc.tensor.transpose(coT[:, :], cosb[:, :], idn32_t[:, :])
        nc.vector.tensor_copy(cf_t[:, :], coT[:, :])

        at_full = [atfp.tile([128, 128], DBF, tag=f"atf{i}", name=f"atf{i}")
                   for i in range(2)]
        for t_ in at_full:
            nc.vector.memset(t_[0:64, 64:128], 0.0)
            nc.vector.memset(t_[64:128, 0:64], 0.0)

        for t in range(16):  # pairs of patches stacked on partitions
            phl2 = p2p.tile([128, 2, 16, 128], DBF, tag="phl2", name="phl2")
            nc.sync.dma_start(
                out=phl2[:, :, :, :],
                in_=phl2t.rearrange("t q (s c n) -> t q s c n", s=2, c=16)[t])
            ph2 = phl2[:, 0, :, :]
            pl2 = phl2[:, 1, :, :]
            sps = psB.tile([128, 64], F32, tag="sps", name="sps", bufs=3)
            # bk offset broadcast over all 128 score rows (f32, exact)
            nc.tensor.matmul(sps[:, :], ones1[0:1, :], cf_t[0:1, :],
                             start=True, stop=False)
            # S^T = P^T M2, 3-term hi/lo
            for cc in range(16):
                for ti, (pp_, mm_) in enumerate(
                        [(ph2[:, cc, :], m2h), (pl2[:, cc, :], m2h),
                         (ph2[:, cc, :], m2l)]):
                    nc.tensor.matmul(
                        sps[:, :],
                        pp_,
                        mm_[:, cc, :],
                        start=False,
                        stop=(cc == 15 and ti == 2))
            negmax = str2.tile([128, 1], F32, tag="negmax", name="negmax")
            nc.vector.tensor_reduce(negmax[:, :], sps[:, :], axis=AX.X, op=ALU.max,
                                    negate=True)
            e_sb = str2.tile([128, 64], F32, tag="esb", name="esb")
            ssum = str2.tile([128, 1], F32, tag="ssum", name="ssum")
            nc.scalar.activation(e_sb[:, :], sps[:, :], AF.Exp,
                                 bias=negmax[:, :], scale=1.0, accum_out=ssum[:, :])
            rec = str2.tile([128, 1], F32, tag="rec", name="rec")
            nc.vector.reciprocal(rec[:, :], ssum[:, :])
            atf = at_full[t % 2]
            nc.vector.tensor_scalar(atf[0:64, 0:64], e_sb[0:64, :],
                                    rec[0:64, :], None, op0=ALU.mult)
            nc.vector.tensor_scalar(atf[64:128, 64:128], e_sb[64:128, :],
                                    rec[64:128, :], None, op0=ALU.mult)
            abP = psB.tile([128, 128], DBF, tag="abP", name="abP", bufs=2)
            nc.tensor.transpose(abP[:, :], atf[:, :], idn_t[:, :])
            nc.vector.tensor_copy(a_blk[:, t, :], abP[:, :])

    # ================= phase 3+4 fused per 512-col block ====================
    with (
        tc.tile_pool(name="xrp", bufs=2) as xrp,
        tc.tile_pool(name="zbp", bufs=2) as zbp,
        tc.tile_pool(name="str4", bufs=4) as str4,
        tc.tile_pool(name="psC", bufs=2, space="PSUM") as psC,
    ):
        xr_r = xr2.rearrange("g q (m j) -> g q m j", m=16)
        for g in range(4):
            if g == 0:
                pt_t = pt0
            else:
                pt_t = ptp.tile([128, 16, 4, 128], DBF, tag="pt", name="pt")
                nc.gpsimd.dma_start(out=pt_t[:, :, :, :], in_=pt_r[g])
            xr_t = xrp.tile([128, 16, 512], DBF, tag="xr", name="xr")
            nc.sync.dma_start(out=xr_t[:, :, :], in_=xr_r[g])
            z_blk = zbp.tile([128, 16, 512], DBF, tag="zb", name="zb")
            for q in range(16):
                zps = psC.tile([128, 512], F32, tag="zps", name="zps")
                for pr in range(4):
                    nc.tensor.matmul(
                        zps[:, 128 * pr:128 * (pr + 1)],
                        pt_t[:, q, pr, :],
                        a_blk[:, 4 * g + pr, :],
                        start=True, stop=True)
                nc.scalar.activation(z_blk[:, q, :], zps[:, :], AF.Copy,
                                     bias=0.0, scale=1.0)
            for m in range(16):
                ops = psC.tile([128, 512], F32, tag="ops", name="ops")
                for q in range(16):
                    nc.tensor.matmul(
                        ops[:, :],
                        wv_t[:, q, m, :],
                        z_blk[:, q, :],
                        start=(q == 0), stop=(q == 15))
                o_sb = str4.tile([128, 512], DBF, tag="osb", name="osb")
                nc.vector.tensor_tensor(o_sb[:, :], ops[:, :],
                                        xr_t[:, m, :], op=ALU.add)
                nc.scalar.dma_start(
                    out=out[:, 2048 * m + 512 * g:2048 * m + 512 * (g + 1)],
                    in_=o_sb[:, :])


# ---------------------------------------------------------------------------
# host wrapper
# ---------------------------------------------------------------------------

def _split_bf16x2(a):
    """Split f32 into (hi, lo) bf16 pair with hi + lo ~= a."""
    a = np.asarray(a, np.float32)
    hi = a.astype(BF16)
    lo = (a - hi.astype(np.float32)).astype(BF16)
    return hi, lo


_NC_CACHE = None


def _get_nc():
    global _NC_CACHE
    if _NC_CACHE is None:
        _NC_CACHE = build_program()
    return _NC_CACHE


def make_in_maps(x, Wk, bk, Wq, bq, Wv, bv, inc_W, inc_b,
                 bn_gamma, bn_beta, bn_mean, bn_var):
    x = np.ascontiguousarray(x, dtype=np.float32)
    bns = (bn_gamma / np.sqrt(bn_var + 1e-5)).astype(np.float32)
    bnt = ((inc_b - bn_mean) * bns + bn_beta).astype(np.float32)

    def _wblock(w):
        # [K, C] f32 -> [128, 16*2*K] bf16 hi/lo pre-blocked SBUF image
        wt = np.ascontiguousarray(w.T, dtype=np.float32)        # [C, K]
        hi, lo = _split_bf16x2(wt)
        hl = np.ascontiguousarray(np.stack([hi, lo], axis=1))   # [C, 2, K]
        return np.ascontiguousarray(
            hl.reshape(16, 128, 2, wt.shape[1]).transpose(1, 0, 2, 3)
            .reshape(128, -1))
    wq_h = _wblock(Wq)
    wi_h = _wblock(inc_W)
    wk3_h = np.ascontiguousarray(
        np.asarray(Wk, np.float32).reshape(2, 128, 16, 128)
        .transpose(1, 0, 2, 3).reshape(128, 4096))
    # wv: [128 c', (q 16, m 16, k' 128)]
    wv_h = np.ascontiguousarray(
        np.ascontiguousarray(Wv.T).reshape(16, 128, 16, 128)
        .transpose(1, 0, 2, 3).reshape(128, 32768)).astype(BF16)
    idn_h = np.eye(128, dtype=BF16)
    idn32_h = np.eye(64, dtype=np.float32)

    in_maps = []
    for core in range(N_CORES):
        b, half = core // 2, core % 2
        xb = x[b]
        pa = (xb.reshape(C, 8, 8, 8, 8).transpose(1, 3, 2, 4, 0)
              .reshape(64, C, 64))                        # [n, c_new, y]
        ploc = pa[32 * half:32 * half + 32]               # [32, 2048, 64]
        p2d = np.ascontiguousarray(
            ploc.transpose(1, 0, 2).reshape(C, 2048))     # [c_new, (n,y)]
        p_hi, p_lo = _split_bf16x2(p2d)
        def _pblock(p):
            return np.ascontiguousarray(
                p.reshape(16, 128, 4, 512).transpose(2, 1, 0, 3)
                .reshape(4, 128, 8192))
        ph_h = _pblock(p_hi)
        pl_h = _pblock(p_lo)
        # phl2t: per-pair contiguous re-stream layout [t, 128, (hi/lo, cc, 128)]
        def _tblock(p):
            return (np.asarray(p).reshape(16, 128, 16, 128)
                    .transpose(2, 1, 0, 3).reshape(16, 128, 2048))
        phl2t_h = np.ascontiguousarray(
            np.concatenate([_tblock(p_hi), _tblock(p_lo)], axis=2))
        # pt: [g, (s,x), (q, pr, c)]
        pt_h = np.ascontiguousarray(
            ploc.reshape(4, 4, 2, 16, 128, 64)            # [g, pr, s, q, c, x]
            .transpose(0, 2, 5, 3, 1, 4).reshape(4, 128, 8192)).astype(BF16)
        xr2d = (xb[:, 32 * half:32 * half + 32, :].reshape(C, 2048)
                + bv[:, None]).astype(np.float32)
        xr_h = np.ascontiguousarray(
            xr2d.reshape(16, 128, 4, 512).transpose(2, 1, 0, 3)
            .reshape(4, 128, 8192)).astype(BF16)
        xi2d = np.ascontiguousarray(
            xb[:, 8 * half:8 * half + 8, :].reshape(C, 512))
        xi_hi, xi_lo = _split_bf16x2(xi2d)
        def _xiblock(p):
            return np.ascontiguousarray(
                p.reshape(16, 128, 512).transpose(1, 0, 2).reshape(128, 8192))
        in_maps.append({
            "ph": ph_h, "pl": pl_h, "phl2t": phl2t_h,
            "xih": _xiblock(xi_hi),
            "xil": _xiblock(xi_lo), "pt2": pt_h, "xr2": xr_h,
            "wq2": wq_h, "wi2": wi_h, "wk3": wk3_h, "wv2": wv_h,
            "bk": np.asarray(bk, np.float32), "bq": np.asarray(bq, np.float32),
            "bns": bns, "bnt": bnt, "idn": idn_h, "idn32": idn32_h,
        })
    return in_maps


def assemble_output(res):
    out = np.empty((4, C, 64, 64), dtype=np.float32)
    for core in range(N_CORES):
        b, half = core // 2, core % 2
        o = np.asarray(res.results[core]["out"]).reshape(128, 16, 4, 512)
        o2 = (o.astype(np.float32).transpose(1, 0, 2, 3).reshape(C, 2048))
        out[b, :, 32 * half:32 * half + 32, :] = o2.reshape(C, 32, 64)
    return out


def kernel(**inputs):
    nc = _get_nc()
    in_maps = make_in_maps(**inputs)
    res = run_bass_kernel_spmd(nc, in_maps, list(range(N_CORES)))
    return assemble_output(res)
